# revision 7
# baseline (speedup 1.0000x reference)
"""Trainium2 Bass kernel for conv3x3(valid) + bias + maxpool2x2 + relu.

Problem: x[64,1,512,512] f32, kernels[5,1,3,3], biases[5]
  -> out[64,5,255,255] f32.

Distribution: pure data parallel over 8 cores, 8 images per core.

Per-core algorithm:
- Conv as PE matmul in fp16 (host-rounded): contract dim K = 3
  column-shifted groups x 42 image rows. The 3 dj taps live in K via the
  shifted groups; the 3 di taps live in a banded lhsT within each group.
  One matmul covers all 9 taps for 20 output rows x 5 channels (M=100)
  per row-parity. Even/odd conv rows go to separate PSUM banks at the
  same partition range so the H-direction pool is an elementwise max.
- Pooling/bias/relu (superbands processed in pairs to amortize op
  overheads):
    ACT: rE = relu(psE + bias)            (evacuates parity E)
    DVE: H  = max(psO + bias, rE)         (scalar_tensor_tensor, 1 PSUM in)
    POOL: out = max(H[0::2], H[1::2])     (W-pool on stride-2 pairs)
- DMA: few large DMAs. Input loaded as three column-shifted group copies
  straight from HBM in fp16 with multi-band/multi-image access patterns;
  40-row non-overlapping blocks plus a tiny 2-row fixup DMA provide each
  band's 42-row window without overlapping source dims.
"""

import numpy as np

N_CORES = 8
IMG_PER_CORE = 8
HW = 512
POOLED = 255
CONVW = 510

P_FULL = 20          # pooled rows per full superband
B_FULL = 12          # full superbands per image (12*20 + 15 = 255)
P_TAIL = 15
R_FULL = 42          # input rows per full superband (2*20+2)
R_TAIL = 32
IMGS_PER_CHUNK = 2
N_CHUNK = IMG_PER_CORE // IMGS_PER_CHUNK
BLK = B_FULL * IMGS_PER_CHUNK   # 24 column blocks in the chunk X tile

_CACHE: dict = {}


def _build_lhsT(w: np.ndarray, parity: int, p_rows: int, in_rows: int) -> np.ndarray:
    """lhsT[g*in_rows + r, c*p_rows + io] = w[c, r-(2*io+parity), g]."""
    K = 3 * in_rows
    M = 5 * p_rows
    lhsT = np.zeros((K, M), dtype=np.float32)
    for g in range(3):
        for c in range(5):
            for io in range(p_rows):
                t = 2 * io + parity
                for dr in range(3):
                    r = t + dr
                    if r < in_rows:
                        lhsT[g * in_rows + r, c * p_rows + io] = w[c, dr, g]
    return lhsT


def _build_program():
    import concourse.bacc as bacc
    import concourse.tile as tile
    from concourse import mybir
    from concourse.alu_op_type import AluOpType

    F32 = mybir.dt.float32
    F16 = mybir.dt.float16
    Relu = mybir.ActivationFunctionType.Relu

    nc = bacc.Bacc(trn_type="TRN2", target_bir_lowering=False, debug=False)

    x_ap = nc.dram_tensor("x", [IMG_PER_CORE, HW, HW], F16, kind="ExternalInput").ap()
    ltE = nc.dram_tensor("ltE", [126, 100], F16, kind="ExternalInput").ap()
    ltO = nc.dram_tensor("ltO", [126, 100], F16, kind="ExternalInput").ap()
    ltEt = nc.dram_tensor("ltEt", [96, 75], F16, kind="ExternalInput").ap()
    ltOt = nc.dram_tensor("ltOt", [96, 75], F16, kind="ExternalInput").ap()
    biasP = nc.dram_tensor("biasP", [100, 1], F32, kind="ExternalInput").ap()
    biasPt = nc.dram_tensor("biasPt", [75, 1], F32, kind="ExternalInput").ap()
    # Permuted output layouts so the big output DMAs balance to <=3 dims;
    # host unpermutes. outA[c, io, i, jo, b] covers pooled rows b*20+io<240;
    # outB[c, io, i, jo] covers pooled rows 240+io.
    outA = nc.dram_tensor(
        "outA", [5, P_FULL, IMG_PER_CORE, POOLED, B_FULL], F32, kind="ExternalOutput"
    ).ap()
    outB = nc.dram_tensor(
        "outB", [5, P_TAIL, IMG_PER_CORE, POOLED], F32, kind="ExternalOutput"
    ).ap()

    with tile.TileContext(nc) as tc:
        with (
            tc.tile_pool(name="consts", bufs=1) as consts,
            tc.tile_pool(name="xc", bufs=2) as xpool,
            tc.tile_pool(name="xtail", bufs=1) as xtpool,
            tc.tile_pool(name="psE", bufs=2, space="PSUM") as psEp,
            tc.tile_pool(name="psO", bufs=2, space="PSUM") as psOp,
            tc.tile_pool(name="rE", bufs=3) as repool,
            tc.tile_pool(name="H", bufs=3) as hpool,
            tc.tile_pool(name="ot", bufs=2) as opool,
            tc.tile_pool(name="ott", bufs=1) as otpool,
        ):
            wE = consts.tile([126, 100], F16, tag="wE")
            wO = consts.tile([126, 100], F16, tag="wO")
            wEt = consts.tile([96, 75], F16, tag="wEt")
            wOt = consts.tile([96, 75], F16, tag="wOt")
            bt = consts.tile([100, 1], F32, tag="bt")
            btt = consts.tile([75, 1], F32, tag="btt")
            nc.sync.dma_start(wE[:], ltE[:])
            nc.sync.dma_start(wO[:], ltO[:])
            nc.sync.dma_start(wEt[:], ltEt[:])
            nc.sync.dma_start(wOt[:], ltOt[:])
            nc.sync.dma_start(bt[:], biasP[:])
            nc.sync.dma_start(btt[:], biasPt[:])

            # ---- tail staging: all 8 images, rows 480..511, 3 groups ----
            Xt = xtpool.tile([96, HW * IMG_PER_CORE], F16, tag="Xt")
            for g in range(3):
                # src rows 480..511, cols g..g+509, all images; enumerate (r,i,w)
                src = x_ap[:, 480:512, g : g + CONVW].rearrange("i r w -> r i w")
                dst = Xt[32 * g : 32 * g + 32, :].rearrange(
                    "p (i w) -> p i w", i=IMG_PER_CORE
                )[:, :, 0:CONVW]
                nc.sync.dma_start(dst, src)

            ot_t = otpool.tile([75, POOLED * IMG_PER_CORE], F32, tag="ott")

            for ch in range(N_CHUNK):
                i0 = ch * IMGS_PER_CHUNK
                # ---- chunk staging: 2 images x 12 bands x 512 cols, 3 groups
                X = xpool.tile([126, HW * BLK], F16, tag="X")
                for g in range(3):
                    # main blocks: rows 40b..40b+39 -> band rows 0..39
                    for i in range(IMGS_PER_CHUNK):
                        srcm = x_ap[i0 + i, 0:480, g : g + CONVW].rearrange(
                            "(b r) w -> r b w", r=40
                        )
                        dstm = X[42 * g : 42 * g + 40, :].rearrange(
                            "p (i b w) -> p i b w", i=IMGS_PER_CHUNK, b=B_FULL
                        )[:, i, :, 0:CONVW]
                        nc.sync.dma_start(dstm, srcm)
                    # fixup rows 40b+40 and 40b+41 -> band rows 40, 41
                    for fr in (40, 41):
                        srcf = x_ap[
                            i0 : i0 + IMGS_PER_CHUNK, fr : fr + 441 : 40, g : g + CONVW
                        ]
                        dstf = X[42 * g + fr : 42 * g + fr + 1, :].rearrange(
                            "p (i b w) -> p i b w", i=IMGS_PER_CHUNK, b=B_FULL
                        )[:, :, :, 0:CONVW]
                        nc.sync.dma_start(dstf, srcf)

                # free layout (i, jo, b): offset = i*(255*12) + jo*12 + b
                ot = opool.tile([100, POOLED * BLK], F32, tag="ot")
                ot4 = ot[:].rearrange(
                    "p (i jo b) -> p i jo b", i=IMGS_PER_CHUNK, jo=POOLED
                )

                for i in range(IMGS_PER_CHUNK):
                    for pb in range(B_FULL // 2):
                        b0 = 2 * pb
                        psE = psEp.tile([100, 1024], F32, tag="psE")
                        psO = psOp.tile([100, 1024], F32, tag="psO")
                        for j, b in enumerate((b0, b0 + 1)):
                            base = HW * (i * B_FULL + b)
                            rhs = X[0:126, base : base + CONVW]
                            nc.tensor.matmul(
                                psE[:, 512 * j : 512 * j + CONVW],
                                wE[:, :], rhs, start=True, stop=True,
                            )
                            nc.tensor.matmul(
                                psO[:, 512 * j : 512 * j + CONVW],
                                wO[:, :], rhs, start=True, stop=True,
                            )
                        psEv = psE[:].rearrange("p (k w) -> p k w", k=2)[:, :, 0:CONVW]
                        psOv = psO[:].rearrange("p (k w) -> p k w", k=2)[:, :, 0:CONVW]
                        rE = repool.tile([100, 1024], F32, tag="rE")
                        rEv = rE[:].rearrange("p (k w) -> p k w", k=2)[:, :, 0:CONVW]
                        nc.scalar.activation(rEv, psEv, Relu, bias=bt[:], scale=1.0)
                        Ht = hpool.tile([100, 1024], F32, tag="H")
                        Hv = Ht[:].rearrange("p (k w) -> p k w", k=2)[:, :, 0:CONVW]
                        nc.vector.scalar_tensor_tensor(
                            Hv, psOv, bt[:], rEv,
                            op0=AluOpType.add, op1=AluOpType.max,
                        )
                        He = Ht[:].rearrange("p (k w) -> p k w", k=2)[:, :, 0:CONVW:2]
                        Ho = Ht[:].rearrange("p (k w) -> p k w", k=2)[:, :, 1:CONVW:2]
                        ov = ot4[:, i, :, b0 : b0 + 2].rearrange(
                            "p jo k -> p k jo"
                        )
                        nc.gpsimd.tensor_tensor(ov, He, Ho, op=AluOpType.max)

                # ---- chunk output DMA: (c, io, i, jo, b) enumeration ----
                osrc = ot[:, :]
                odst = outA[:, :, i0 : i0 + IMGS_PER_CHUNK, :, :].rearrange(
                    "c io i jo b -> (c io) (i jo b)"
                )
                nc.scalar.dma_start(odst, osrc)

                # ---- tails for the chunk's images ----
                for i in range(IMGS_PER_CHUNK):
                    img = i0 + i
                    psEt = psEp.tile([100, 1024], F32, tag="psE")
                    psOt = psOp.tile([100, 1024], F32, tag="psO")
                    rhs_t = Xt[0:96, :].rearrange(
                        "p (i w) -> p i w", i=IMG_PER_CORE
                    )[:, img, 0:CONVW]
                    nc.tensor.matmul(
                        psEt[0:75, 0:CONVW], wEt[:, :], rhs_t, start=True, stop=True
                    )
                    nc.tensor.matmul(
                        psOt[0:75, 0:CONVW], wOt[:, :], rhs_t, start=True, stop=True
                    )
                    rEt = repool.tile([100, 1024], F32, tag="rE")
                    nc.scalar.activation(
                        rEt[0:75, 0:CONVW], psEt[0:75, 0:CONVW], Relu,
                        bias=btt[:], scale=1.0,
                    )
                    Htt = hpool.tile([100, 1024], F32, tag="H")
                    nc.vector.scalar_tensor_tensor(
                        Htt[0:75, 0:CONVW], psOt[0:75, 0:CONVW], btt[:],
                        rEt[0:75, 0:CONVW],
                        op0=AluOpType.add, op1=AluOpType.max,
                    )
                    nc.gpsimd.tensor_tensor(
                        ot_t[:, POOLED * img : POOLED * (img + 1)],
                        Htt[0:75, 0:CONVW:2], Htt[0:75, 1:CONVW:2],
                        op=AluOpType.max,
                    )

            # ---- tail output DMA: all 8 images ----
            tsrc = ot_t[:, :]
            tdst = outB[:].rearrange("c io i jo -> (c io) (i jo)")
            nc.scalar.dma_start(tdst, tsrc)

    nc.compile()
    return nc


def _get_program():
    if "nc" not in _CACHE:
        _CACHE["nc"] = _build_program()
    return _CACHE["nc"]


def _host_inputs(kernels: np.ndarray, biases: np.ndarray):
    w = kernels.reshape(5, 3, 3).astype(np.float32)
    ltE = _build_lhsT(w, 0, P_FULL, R_FULL).astype(np.float16)
    ltO = _build_lhsT(w, 1, P_FULL, R_FULL).astype(np.float16)
    ltEt = _build_lhsT(w, 0, P_TAIL, R_TAIL).astype(np.float16)
    ltOt = _build_lhsT(w, 1, P_TAIL, R_TAIL).astype(np.float16)
    biasP = np.repeat(biases.astype(np.float32), P_FULL).reshape(100, 1)
    biasPt = np.repeat(biases.astype(np.float32), P_TAIL).reshape(75, 1)
    return ltE, ltO, ltEt, ltOt, biasP, biasPt


def kernel(x: np.ndarray, kernels: np.ndarray, biases: np.ndarray) -> np.ndarray:
    from concourse.bass_utils import run_bass_kernel_spmd

    nc = _get_program()
    ltE, ltO, ltEt, ltOt, biasP, biasPt = _host_inputs(
        np.asarray(kernels), np.asarray(biases)
    )
    xh = np.asarray(x, dtype=np.float32).reshape(64, HW, HW).astype(np.float16)

    in_maps = []
    for i in range(N_CORES):
        in_maps.append(
            {
                "x": xh[i * IMG_PER_CORE : (i + 1) * IMG_PER_CORE],
                "ltE": ltE,
                "ltO": ltO,
                "ltEt": ltEt,
                "ltOt": ltOt,
                "biasP": biasP,
                "biasPt": biasPt,
            }
        )
    res = run_bass_kernel_spmd(nc, in_maps, list(range(N_CORES)))
    out = np.empty((64, 5, POOLED, POOLED), dtype=np.float32)
    for i in range(N_CORES):
        # outA [5, 20, 8, 255, 12] -> rows b*20+io
        a = res.results[i]["outA"].transpose(2, 0, 4, 1, 3).reshape(
            IMG_PER_CORE, 5, P_FULL * B_FULL, POOLED
        )
        bpart = res.results[i]["outB"].transpose(2, 0, 1, 3)
        sl = slice(i * IMG_PER_CORE, (i + 1) * IMG_PER_CORE)
        out[sl, :, 0 : P_FULL * B_FULL, :] = a
        out[sl, :, P_FULL * B_FULL :, :] = bpart
    return out


# revision 8
# speedup vs baseline: 3.4041x; 3.4041x over previous
"""Trainium2 Bass kernel for conv3x3(valid) + bias + maxpool2x2 + relu.

Problem: x[64,1,512,512] f32, kernels[5,1,3,3], biases[5]
  -> out[64,5,255,255] f32.

Distribution: pure data parallel over 8 cores, 8 images per core.

Per-core algorithm:
- Conv as PE matmul in fp16 (host-rounded): contract dim K = 3
  column-shifted groups x 42 image rows. The 3 dj taps live in K via the
  shifted groups; the 3 di taps live in a banded lhsT within each group.
  One matmul covers all 9 taps for 20 output rows x 5 channels (M=100)
  per row-parity. Even/odd conv rows go to separate PSUM banks at the
  same partition range so the H-direction pool is an elementwise max.
- Pooling/bias/relu (superbands processed in pairs to amortize op
  overheads):
    ACT: rE = relu(psE + bias)            (evacuates parity E)
    DVE: H  = max(psO + bias, rE)         (scalar_tensor_tensor, 1 PSUM in)
    POOL: out = max(H[0::2], H[1::2])     (W-pool on stride-2 pairs)
- DMA: few large DMAs. Input loaded as three column-shifted group copies
  straight from HBM in fp16 with multi-band/multi-image access patterns;
  40-row non-overlapping blocks plus a tiny 2-row fixup DMA provide each
  band's 42-row window without overlapping source dims.
"""

import numpy as np

N_CORES = 8
IMG_PER_CORE = 8
HW = 512
POOLED = 255
CONVW = 510

P_FULL = 20          # pooled rows per full superband
B_FULL = 12          # full superbands per image (12*20 + 15 = 255)
P_TAIL = 15
R_FULL = 42          # input rows per full superband (2*20+2)
R_TAIL = 32
IMGS_PER_CHUNK = 2
N_CHUNK = IMG_PER_CORE // IMGS_PER_CHUNK
BLK = B_FULL * IMGS_PER_CHUNK   # 24 column blocks in the chunk X tile

_CACHE: dict = {}


def _build_lhsT(w: np.ndarray, parity: int, p_rows: int, in_rows: int) -> np.ndarray:
    """lhsT[g*in_rows + r, c*p_rows + io] = w[c, r-(2*io+parity), g]."""
    K = 3 * in_rows
    M = 5 * p_rows
    lhsT = np.zeros((K, M), dtype=np.float32)
    for g in range(3):
        for c in range(5):
            for io in range(p_rows):
                t = 2 * io + parity
                for dr in range(3):
                    r = t + dr
                    if r < in_rows:
                        lhsT[g * in_rows + r, c * p_rows + io] = w[c, dr, g]
    return lhsT


def _win_ap(x_ap, img, row0, nrows):
    """Source AP [[1,3],[512,nrows],[1,510]] at x[img, row0, 0]: enumerates
    (group-shift g, band row r, col w) with overlapping reads -- the three
    column-shifted K-groups of one band in a single 126-partition DMA."""
    import bass_rust

    c = x_ap.copy()
    c.offset = img * (HW * HW) + row0 * HW
    c.ap = bass_rust.VecI64Pair([[1, 3], [HW, nrows], [1, CONVW]])
    return c


def _build_program():
    import concourse.bacc as bacc
    import concourse.tile as tile
    from concourse import mybir
    from concourse.alu_op_type import AluOpType

    F32 = mybir.dt.float32
    F16 = mybir.dt.float16
    Relu = mybir.ActivationFunctionType.Relu

    nc = bacc.Bacc(trn_type="TRN2", target_bir_lowering=False, debug=False)

    x_ap = nc.dram_tensor("x", [IMG_PER_CORE, HW, HW], F16, kind="ExternalInput").ap()
    ltE = nc.dram_tensor("ltE", [126, 100], F16, kind="ExternalInput").ap()
    ltO = nc.dram_tensor("ltO", [126, 100], F16, kind="ExternalInput").ap()
    ltEt = nc.dram_tensor("ltEt", [96, 75], F16, kind="ExternalInput").ap()
    ltOt = nc.dram_tensor("ltOt", [96, 75], F16, kind="ExternalInput").ap()
    biasP = nc.dram_tensor("biasP", [100, 1], F32, kind="ExternalInput").ap()
    biasPt = nc.dram_tensor("biasPt", [75, 1], F32, kind="ExternalInput").ap()
    # Permuted output layouts so the big output DMAs balance to <=3 dims;
    # host unpermutes. outA[c, io, i, jo, b] covers pooled rows b*20+io<240;
    # outB[c, io, i, jo] covers pooled rows 240+io.
    outA = nc.dram_tensor(
        "outA", [5, P_FULL, IMG_PER_CORE, POOLED, B_FULL], F32, kind="ExternalOutput"
    ).ap()
    outB = nc.dram_tensor(
        "outB", [5, P_TAIL, IMG_PER_CORE, POOLED], F32, kind="ExternalOutput"
    ).ap()

    with tile.TileContext(nc) as tc:
        with (
            tc.tile_pool(name="consts", bufs=1) as consts,
            tc.tile_pool(name="xc", bufs=2) as xpool,
            tc.tile_pool(name="xtail", bufs=1) as xtpool,
            tc.tile_pool(name="psE", bufs=2, space="PSUM") as psEp,
            tc.tile_pool(name="psO", bufs=2, space="PSUM") as psOp,
            tc.tile_pool(name="rE", bufs=3) as repool,
            tc.tile_pool(name="H", bufs=3) as hpool,
            tc.tile_pool(name="ot", bufs=2) as opool,
            tc.tile_pool(name="ott", bufs=1) as otpool,
        ):
            wE = consts.tile([126, 100], F16, tag="wE")
            wO = consts.tile([126, 100], F16, tag="wO")
            wEt = consts.tile([96, 75], F16, tag="wEt")
            wOt = consts.tile([96, 75], F16, tag="wOt")
            bt = consts.tile([100, 1], F32, tag="bt")
            btt = consts.tile([75, 1], F32, tag="btt")
            nc.sync.dma_start(wE[:], ltE[:])
            nc.sync.dma_start(wO[:], ltO[:])
            nc.sync.dma_start(wEt[:], ltEt[:])
            nc.sync.dma_start(wOt[:], ltOt[:])
            nc.sync.dma_start(bt[:], biasP[:])
            nc.sync.dma_start(btt[:], biasPt[:])

            # ---- tail staging: all 8 images, rows 480..511, 3 groups ----
            Xt = xtpool.tile([96, HW * IMG_PER_CORE], F16, tag="Xt")
            for img in range(IMG_PER_CORE):
                nc.sync.dma_start(
                    Xt[0:96, HW * img : HW * img + CONVW],
                    _win_ap(x_ap, img, 480, R_TAIL),
                )

            ot_t = otpool.tile([75, POOLED * IMG_PER_CORE], F32, tag="ott")

            for ch in range(N_CHUNK):
                i0 = ch * IMGS_PER_CHUNK
                # ---- chunk staging: 2 images x 12 bands x 512 cols, 3 groups
                X = xpool.tile([126, HW * BLK], F16, tag="X")
                for i in range(IMGS_PER_CHUNK):
                    for b in range(B_FULL):
                        col = HW * (i * B_FULL + b)
                        nc.sync.dma_start(
                            X[0:126, col : col + CONVW],
                            _win_ap(x_ap, i0 + i, 40 * b, R_FULL),
                        )

                # free layout (i, jo, b): offset = i*(255*12) + jo*12 + b
                ot = opool.tile([100, POOLED * BLK], F32, tag="ot")
                ot4 = ot[:].rearrange(
                    "p (i jo b) -> p i jo b", i=IMGS_PER_CHUNK, jo=POOLED
                )

                for i in range(IMGS_PER_CHUNK):
                    for pb in range(B_FULL // 2):
                        b0 = 2 * pb
                        psE = psEp.tile([100, 1024], F32, tag="psE")
                        psO = psOp.tile([100, 1024], F32, tag="psO")
                        for j, b in enumerate((b0, b0 + 1)):
                            base = HW * (i * B_FULL + b)
                            rhs = X[0:126, base : base + CONVW]
                            nc.tensor.matmul(
                                psE[:, 512 * j : 512 * j + CONVW],
                                wE[:, :], rhs, start=True, stop=True,
                            )
                            nc.tensor.matmul(
                                psO[:, 512 * j : 512 * j + CONVW],
                                wO[:, :], rhs, start=True, stop=True,
                            )
                        psEv = psE[:].rearrange("p (k w) -> p k w", k=2)[:, :, 0:CONVW]
                        psOv = psO[:].rearrange("p (k w) -> p k w", k=2)[:, :, 0:CONVW]
                        rE = repool.tile([100, 1024], F32, tag="rE")
                        rEv = rE[:].rearrange("p (k w) -> p k w", k=2)[:, :, 0:CONVW]
                        nc.scalar.activation(rEv, psEv, Relu, bias=bt[:], scale=1.0)
                        Ht = hpool.tile([100, 1024], F32, tag="H")
                        Hv = Ht[:].rearrange("p (k w) -> p k w", k=2)[:, :, 0:CONVW]
                        nc.vector.scalar_tensor_tensor(
                            Hv, psOv, bt[:], rEv,
                            op0=AluOpType.add, op1=AluOpType.max,
                        )
                        He = Ht[:].rearrange("p (k w) -> p k w", k=2)[:, :, 0:CONVW:2]
                        Ho = Ht[:].rearrange("p (k w) -> p k w", k=2)[:, :, 1:CONVW:2]
                        ov = ot4[:, i, :, b0 : b0 + 2].rearrange(
                            "p jo k -> p k jo"
                        )
                        nc.gpsimd.tensor_tensor(ov, He, Ho, op=AluOpType.max)

                # ---- chunk output DMA: (c, io, i, jo, b) enumeration ----
                osrc = ot[:, :]
                odst = outA[:, :, i0 : i0 + IMGS_PER_CHUNK, :, :].rearrange(
                    "c io i jo b -> (c io) (i jo b)"
                )
                nc.gpsimd.dma_start(odst, osrc)

                # ---- tails for the chunk's images ----
                for i in range(IMGS_PER_CHUNK):
                    img = i0 + i
                    psEt = psEp.tile([100, 1024], F32, tag="psE")
                    psOt = psOp.tile([100, 1024], F32, tag="psO")
                    rhs_t = Xt[0:96, :].rearrange(
                        "p (i w) -> p i w", i=IMG_PER_CORE
                    )[:, img, 0:CONVW]
                    nc.tensor.matmul(
                        psEt[0:75, 0:CONVW], wEt[:, :], rhs_t, start=True, stop=True
                    )
                    nc.tensor.matmul(
                        psOt[0:75, 0:CONVW], wOt[:, :], rhs_t, start=True, stop=True
                    )
                    rEt = repool.tile([100, 1024], F32, tag="rE")
                    nc.scalar.activation(
                        rEt[0:75, 0:CONVW], psEt[0:75, 0:CONVW], Relu,
                        bias=btt[:], scale=1.0,
                    )
                    Htt = hpool.tile([100, 1024], F32, tag="H")
                    nc.vector.scalar_tensor_tensor(
                        Htt[0:75, 0:CONVW], psOt[0:75, 0:CONVW], btt[:],
                        rEt[0:75, 0:CONVW],
                        op0=AluOpType.add, op1=AluOpType.max,
                    )
                    nc.gpsimd.tensor_tensor(
                        ot_t[:, POOLED * img : POOLED * (img + 1)],
                        Htt[0:75, 0:CONVW:2], Htt[0:75, 1:CONVW:2],
                        op=AluOpType.max,
                    )

            # ---- tail output DMA: all 8 images ----
            tsrc = ot_t[:, :]
            tdst = outB[:].rearrange("c io i jo -> (c io) (i jo)")
            nc.scalar.dma_start(tdst, tsrc)

    nc.compile()
    return nc


def _get_program():
    if "nc" not in _CACHE:
        _CACHE["nc"] = _build_program()
    return _CACHE["nc"]


def _host_inputs(kernels: np.ndarray, biases: np.ndarray):
    w = kernels.reshape(5, 3, 3).astype(np.float32)
    ltE = _build_lhsT(w, 0, P_FULL, R_FULL).astype(np.float16)
    ltO = _build_lhsT(w, 1, P_FULL, R_FULL).astype(np.float16)
    ltEt = _build_lhsT(w, 0, P_TAIL, R_TAIL).astype(np.float16)
    ltOt = _build_lhsT(w, 1, P_TAIL, R_TAIL).astype(np.float16)
    biasP = np.repeat(biases.astype(np.float32), P_FULL).reshape(100, 1)
    biasPt = np.repeat(biases.astype(np.float32), P_TAIL).reshape(75, 1)
    return ltE, ltO, ltEt, ltOt, biasP, biasPt


def kernel(x: np.ndarray, kernels: np.ndarray, biases: np.ndarray) -> np.ndarray:
    from concourse.bass_utils import run_bass_kernel_spmd

    nc = _get_program()
    ltE, ltO, ltEt, ltOt, biasP, biasPt = _host_inputs(
        np.asarray(kernels), np.asarray(biases)
    )
    xh = np.asarray(x, dtype=np.float32).reshape(64, HW, HW).astype(np.float16)

    in_maps = []
    for i in range(N_CORES):
        in_maps.append(
            {
                "x": xh[i * IMG_PER_CORE : (i + 1) * IMG_PER_CORE],
                "ltE": ltE,
                "ltO": ltO,
                "ltEt": ltEt,
                "ltOt": ltOt,
                "biasP": biasP,
                "biasPt": biasPt,
            }
        )
    res = run_bass_kernel_spmd(nc, in_maps, list(range(N_CORES)))
    out = np.empty((64, 5, POOLED, POOLED), dtype=np.float32)
    for i in range(N_CORES):
        # outA [5, 20, 8, 255, 12] -> rows b*20+io
        a = res.results[i]["outA"].transpose(2, 0, 4, 1, 3).reshape(
            IMG_PER_CORE, 5, P_FULL * B_FULL, POOLED
        )
        bpart = res.results[i]["outB"].transpose(2, 0, 1, 3)
        sl = slice(i * IMG_PER_CORE, (i + 1) * IMG_PER_CORE)
        out[sl, :, 0 : P_FULL * B_FULL, :] = a
        out[sl, :, P_FULL * B_FULL :, :] = bpart
    return out


# revision 10
# speedup vs baseline: 5.0396x; 1.4805x over previous
"""Trainium2 Bass kernel for conv3x3(valid) + bias + maxpool2x2 + relu.

Problem: x[64,1,512,512] f32, kernels[5,1,3,3], biases[5]
  -> out[64,5,255,255] f32.

Distribution: pure data parallel over 8 cores, 8 images per core.

Per-core algorithm:
- Conv as PE matmul in fp16 (host-rounded): contract dim K = 3
  column-shifted groups x 42 image rows. The 3 dj taps live in K via the
  shifted groups; the 3 di taps live in a banded lhsT within each group.
  One matmul covers all 9 taps for 20 output rows x 5 channels (M=100)
  per row-parity. Even/odd conv rows go to separate PSUM banks at the
  same partition range so the H-direction pool is an elementwise max.
- Pooling/bias/relu (superbands processed in pairs to amortize op
  overheads):
    ACT: rE = relu(psE + bias)            (evacuates parity E)
    DVE: H  = max(psO + bias, rE)         (scalar_tensor_tensor, 1 PSUM in)
    POOL: out = max(H[0::2], H[1::2])     (W-pool on stride-2 pairs)
- DMA: few large DMAs. Input loaded as three column-shifted group copies
  straight from HBM in fp16 with multi-band/multi-image access patterns;
  40-row non-overlapping blocks plus a tiny 2-row fixup DMA provide each
  band's 42-row window without overlapping source dims.
"""

import numpy as np

N_CORES = 8
IMG_PER_CORE = 8
HW = 512
POOLED = 255
CONVW = 510

P_FULL = 20          # pooled rows per full superband
B_FULL = 12          # full superbands per image (12*20 + 15 = 255)
P_TAIL = 15
R_FULL = 42          # input rows per full superband (2*20+2)
R_TAIL = 32
IMGS_PER_CHUNK = 2
N_CHUNK = IMG_PER_CORE // IMGS_PER_CHUNK
BLK = B_FULL * IMGS_PER_CHUNK   # 24 column blocks in the chunk X tile

_CACHE: dict = {}


def _build_lhsT(w: np.ndarray, parity: int, p_rows: int, in_rows: int) -> np.ndarray:
    """lhsT[g*in_rows + r, c*p_rows + io] = w[c, r-(2*io+parity), g]."""
    K = 3 * in_rows
    M = 5 * p_rows
    lhsT = np.zeros((K, M), dtype=np.float32)
    for g in range(3):
        for c in range(5):
            for io in range(p_rows):
                t = 2 * io + parity
                for dr in range(3):
                    r = t + dr
                    if r < in_rows:
                        lhsT[g * in_rows + r, c * p_rows + io] = w[c, dr, g]
    return lhsT


def _win_ap(x_ap, img, row0, nrows):
    """Source AP [[1,3],[512,nrows],[1,510]] at x[img, row0, 0]: enumerates
    (group-shift g, band row r, col w) with overlapping reads -- the three
    column-shifted K-groups of one band in a single 126-partition DMA."""
    import bass_rust

    c = x_ap.copy()
    c.offset = img * (HW * HW) + row0 * HW
    c.ap = bass_rust.VecI64Pair([[1, 3], [HW, nrows], [1, CONVW]])
    return c


def _build_program():
    import concourse.bacc as bacc
    import concourse.tile as tile
    from concourse import mybir
    from concourse.alu_op_type import AluOpType

    F32 = mybir.dt.float32
    F16 = mybir.dt.float16
    Relu = mybir.ActivationFunctionType.Relu

    nc = bacc.Bacc(trn_type="TRN2", target_bir_lowering=False, debug=False)

    x_ap = nc.dram_tensor("x", [IMG_PER_CORE, HW, HW], F16, kind="ExternalInput").ap()
    ltE = nc.dram_tensor("ltE", [126, 100], F16, kind="ExternalInput").ap()
    ltO = nc.dram_tensor("ltO", [126, 100], F16, kind="ExternalInput").ap()
    ltEt = nc.dram_tensor("ltEt", [96, 75], F16, kind="ExternalInput").ap()
    ltOt = nc.dram_tensor("ltOt", [96, 75], F16, kind="ExternalInput").ap()
    biasP = nc.dram_tensor("biasP", [100, 1], F32, kind="ExternalInput").ap()
    biasPt = nc.dram_tensor("biasPt", [75, 1], F32, kind="ExternalInput").ap()
    # Permuted output layouts so the big output DMAs balance to <=3 dims;
    # host unpermutes. outA[c, io, i, jo, b] covers pooled rows b*20+io<240;
    # outB[c, io, i, jo] covers pooled rows 240+io.
    outA = nc.dram_tensor(
        "outA", [5, P_FULL, IMG_PER_CORE, B_FULL // 2, 2, POOLED], F32,
        kind="ExternalOutput"
    ).ap()
    outB = nc.dram_tensor(
        "outB", [5, P_TAIL, IMG_PER_CORE, POOLED], F32, kind="ExternalOutput"
    ).ap()

    with tile.TileContext(nc) as tc:
        with (
            tc.tile_pool(name="consts", bufs=1) as consts,
            tc.tile_pool(name="xc", bufs=2) as xpool,
            tc.tile_pool(name="xtail", bufs=1) as xtpool,
            tc.tile_pool(name="psE", bufs=2, space="PSUM") as psEp,
            tc.tile_pool(name="psO", bufs=2, space="PSUM") as psOp,
            tc.tile_pool(name="rE", bufs=3) as repool,
            tc.tile_pool(name="H", bufs=3) as hpool,
            tc.tile_pool(name="ot", bufs=2) as opool,
            tc.tile_pool(name="ott", bufs=1) as otpool,
        ):
            wE = consts.tile([126, 100], F16, tag="wE")
            wO = consts.tile([126, 100], F16, tag="wO")
            wEt = consts.tile([96, 75], F16, tag="wEt")
            wOt = consts.tile([96, 75], F16, tag="wOt")
            bt = consts.tile([100, 1], F32, tag="bt")
            btt = consts.tile([75, 1], F32, tag="btt")
            nc.sync.dma_start(wE[:], ltE[:])
            nc.sync.dma_start(wO[:], ltO[:])
            nc.sync.dma_start(wEt[:], ltEt[:])
            nc.sync.dma_start(wOt[:], ltOt[:])
            nc.sync.dma_start(bt[:], biasP[:])
            nc.sync.dma_start(btt[:], biasPt[:])

            Xt = xtpool.tile([96, HW * IMG_PER_CORE], F16, tag="Xt")
            ot_t = otpool.tile([75, POOLED * IMG_PER_CORE], F32, tag="ott")

            for ch in range(N_CHUNK):
                i0 = ch * IMGS_PER_CHUNK
                # ---- chunk staging: 2 images x 12 bands x 512 cols, 3 groups
                X = xpool.tile([126, HW * BLK], F16, tag="X")
                for i in range(IMGS_PER_CHUNK):
                    for b in range(B_FULL):
                        col = HW * (i * B_FULL + b)
                        nc.sync.dma_start(
                            X[0:126, col : col + CONVW],
                            _win_ap(x_ap, i0 + i, 40 * b, R_FULL),
                        )
                if ch == 0:
                    # tail staging: all 8 images, rows 480..511 (overlaps compute)
                    for img in range(IMG_PER_CORE):
                        nc.sync.dma_start(
                            Xt[0:96, HW * img : HW * img + CONVW],
                            _win_ap(x_ap, img, 480, R_TAIL),
                        )

                # free layout (i, pb, k, jo): contiguous per band-pair
                ot = opool.tile([100, POOLED * BLK], F32, tag="ot")

                for i in range(IMGS_PER_CHUNK):
                    for pb in range(B_FULL // 2):
                        b0 = 2 * pb
                        psE = psEp.tile([100, 1024], F32, tag="psE")
                        psO = psOp.tile([100, 1024], F32, tag="psO")
                        for j, b in enumerate((b0, b0 + 1)):
                            base = HW * (i * B_FULL + b)
                            rhs = X[0:126, base : base + CONVW]
                            nc.tensor.matmul(
                                psE[:, 512 * j : 512 * j + CONVW],
                                wE[:, :], rhs, start=True, stop=True,
                            )
                            nc.tensor.matmul(
                                psO[:, 512 * j : 512 * j + CONVW],
                                wO[:, :], rhs, start=True, stop=True,
                            )
                        psEv = psE[:].rearrange("p (k w) -> p k w", k=2)[:, :, 0:CONVW]
                        psOv = psO[:].rearrange("p (k w) -> p k w", k=2)[:, :, 0:CONVW]
                        rE = repool.tile([100, 1024], F32, tag="rE")
                        rEv = rE[:].rearrange("p (k w) -> p k w", k=2)[:, :, 0:CONVW]
                        nc.scalar.activation(rEv, psEv, Relu, bias=bt[:], scale=1.0)
                        Ht = hpool.tile([100, 1024], F32, tag="H")
                        Hv = Ht[:].rearrange("p (k w) -> p k w", k=2)[:, :, 0:CONVW]
                        nc.vector.scalar_tensor_tensor(
                            Hv, psOv, bt[:], rEv,
                            op0=AluOpType.add, op1=AluOpType.max,
                        )
                        He = Ht[:].rearrange("p (k w) -> p k w", k=2)[:, :, 0:CONVW:2]
                        Ho = Ht[:].rearrange("p (k w) -> p k w", k=2)[:, :, 1:CONVW:2]
                        base = 2 * POOLED * (i * (B_FULL // 2) + pb)
                        ov = ot[:, base : base + 2 * POOLED].rearrange(
                            "p (k jo) -> p k jo", k=2
                        )
                        nc.gpsimd.tensor_tensor(ov, He, Ho, op=AluOpType.max)
                        # per-pair output DMA (interleaves with later TTs)
                        odst = outA[:, :, i0 + i, pb, :, :].rearrange(
                            "c io k jo -> (c io) (k jo)"
                        )
                        nc.gpsimd.dma_start(odst, ot[:, base : base + 2 * POOLED])

                # ---- tails for the chunk's images ----
                for i in range(IMGS_PER_CHUNK):
                    img = i0 + i
                    psEt = psEp.tile([100, 1024], F32, tag="psE")
                    psOt = psOp.tile([100, 1024], F32, tag="psO")
                    rhs_t = Xt[0:96, :].rearrange(
                        "p (i w) -> p i w", i=IMG_PER_CORE
                    )[:, img, 0:CONVW]
                    nc.tensor.matmul(
                        psEt[0:75, 0:CONVW], wEt[:, :], rhs_t, start=True, stop=True
                    )
                    nc.tensor.matmul(
                        psOt[0:75, 0:CONVW], wOt[:, :], rhs_t, start=True, stop=True
                    )
                    rEt = repool.tile([100, 1024], F32, tag="rE")
                    nc.scalar.activation(
                        rEt[0:75, 0:CONVW], psEt[0:75, 0:CONVW], Relu,
                        bias=btt[:], scale=1.0,
                    )
                    Htt = hpool.tile([100, 1024], F32, tag="H")
                    nc.vector.scalar_tensor_tensor(
                        Htt[0:75, 0:CONVW], psOt[0:75, 0:CONVW], btt[:],
                        rEt[0:75, 0:CONVW],
                        op0=AluOpType.add, op1=AluOpType.max,
                    )
                    nc.gpsimd.tensor_tensor(
                        ot_t[:, POOLED * img : POOLED * (img + 1)],
                        Htt[0:75, 0:CONVW:2], Htt[0:75, 1:CONVW:2],
                        op=AluOpType.max,
                    )
                    nc.scalar.dma_start(
                        outB[:, :, img, :].rearrange("c io jo -> (c io) jo"),
                        ot_t[:, POOLED * img : POOLED * (img + 1)],
                    )

    nc.compile()
    return nc


def _get_program():
    if "nc" not in _CACHE:
        _CACHE["nc"] = _build_program()
    return _CACHE["nc"]


def _host_inputs(kernels: np.ndarray, biases: np.ndarray):
    w = kernels.reshape(5, 3, 3).astype(np.float32)
    ltE = _build_lhsT(w, 0, P_FULL, R_FULL).astype(np.float16)
    ltO = _build_lhsT(w, 1, P_FULL, R_FULL).astype(np.float16)
    ltEt = _build_lhsT(w, 0, P_TAIL, R_TAIL).astype(np.float16)
    ltOt = _build_lhsT(w, 1, P_TAIL, R_TAIL).astype(np.float16)
    biasP = np.repeat(biases.astype(np.float32), P_FULL).reshape(100, 1)
    biasPt = np.repeat(biases.astype(np.float32), P_TAIL).reshape(75, 1)
    return ltE, ltO, ltEt, ltOt, biasP, biasPt


def kernel(x: np.ndarray, kernels: np.ndarray, biases: np.ndarray) -> np.ndarray:
    from concourse.bass_utils import run_bass_kernel_spmd

    nc = _get_program()
    ltE, ltO, ltEt, ltOt, biasP, biasPt = _host_inputs(
        np.asarray(kernels), np.asarray(biases)
    )
    xh = np.asarray(x, dtype=np.float32).reshape(64, HW, HW).astype(np.float16)

    in_maps = []
    for i in range(N_CORES):
        in_maps.append(
            {
                "x": xh[i * IMG_PER_CORE : (i + 1) * IMG_PER_CORE],
                "ltE": ltE,
                "ltO": ltO,
                "ltEt": ltEt,
                "ltOt": ltOt,
                "biasP": biasP,
                "biasPt": biasPt,
            }
        )
    res = run_bass_kernel_spmd(nc, in_maps, list(range(N_CORES)))
    out = np.empty((64, 5, POOLED, POOLED), dtype=np.float32)
    for i in range(N_CORES):
        # outA [5, 20, 8, 255, 12] -> rows b*20+io
        a = res.results[i]["outA"].transpose(2, 0, 3, 4, 1, 5).reshape(
            IMG_PER_CORE, 5, P_FULL * B_FULL, POOLED
        )
        bpart = res.results[i]["outB"].transpose(2, 0, 1, 3)
        sl = slice(i * IMG_PER_CORE, (i + 1) * IMG_PER_CORE)
        out[sl, :, 0 : P_FULL * B_FULL, :] = a
        out[sl, :, P_FULL * B_FULL :, :] = bpart
    return out
